# revision 1
# baseline (speedup 1.0000x reference)
# Trainium2 Bass kernel for:
#   q = x @ Wq.T + bq ; k = x @ Wk.T + bk ; v = x @ Wv.T + bv
#   g = sigmoid(x @ Wg.T + bg)
#   out = q * cumsum(k*v, axis=seq) * g
#
# Sharding: tensor-parallel split of the 2048 output features across the 8
# cores (256 features each). All ops are per-feature except the d-contraction
# (each core uses the full x) and the cumsum along seq (handled fully on-core
# per (batch, feature)) -> zero cross-core communication.
#
# On-core layout is [e, t] (features on partitions, tokens on the free dim):
#   - linears:  psum[e,t] += W_chunk.T @ x_chunk   (bf16 matmuls, fp32 accum)
#   - bias:     ACT activation Identity with per-partition bias (bf16 out)
#   - sigmoid:  ACT activation with per-partition bias (bf16 out)
#   - cumsum:   DVE tensor_tensor_scan along the free dim (fp32 state/out),
#               chained across token (sub)tiles via initial=prev[:, -1:]
#   - qg mul on the Pool engine, kv/out muls on DVE.
# The host pre-packs x into unit tiles [B, NU, 128p, KC, TT] (loaded in
# 4-chunk quarters) and W into [128p, KC, E] so every DMA row is one long
# contiguous packet (4KB for x quarters, 2KB for W quarters), and transposes
# the [B, E, S] per-core bf16 outputs back to fp32 at the end. The final unit
# is processed in 128-token sub-tiles to shorten the post-matmul drain chain.

from contextlib import ExitStack

import numpy as np
import ml_dtypes

import concourse.bass as bass  # noqa: F401  (bass types referenced via tile/bacc)
import concourse.tile as tile
from concourse import bacc, mybir
from concourse.bass_utils import run_bass_kernel_spmd

N_CORES = 8
B, S, D = 4, 4096, 2048
E = D // N_CORES  # 256 output features per core
TT = 512          # token tile (free dim of psum)
KC = D // 128     # contraction chunks
XJ = 4            # chunks per x quarter-tile
NXJ = KC // XJ    # x quarter-tiles per unit
NU = S // TT      # token tiles per batch
MH = E // 128     # feature halves (psum groups per linear)
MM_DT = mybir.dt.bfloat16
MM_NP = ml_dtypes.bfloat16


def build_nc(b=B, s=S, d=D, e=E, tt=TT, mm_dt=MM_DT, n_cores=N_CORES):
    kc = KC
    nu = NU
    mh = MH
    f32 = mybir.dt.float32
    names = "qkvg"

    nc = bacc.Bacc(
        "TRN2", target_bir_lowering=False, debug=False, num_devices=n_cores
    )
    # x packed on host: X5[b, n, p, j, c, t] = x[b, n*tt+t, (j*4+c)*128+p]
    # (partition-major so a whole unit [128, kc, tt] is one contiguous-row DMA)
    X5 = nc.dram_tensor(
        "X5", [b, nu, 128, NXJ, XJ, tt], mm_dt, kind="ExternalInput"
    ).ap()
    # W packed on host: W5[p, c, e] = W[core_sl][e, c*128+p]
    W5 = {
        x_: nc.dram_tensor(f"W{x_}5", [128, kc, e], mm_dt, kind="ExternalInput").ap()
        for x_ in names
    }
    bias = {
        x_: nc.dram_tensor(f"b{x_}", [e], f32, kind="ExternalInput").ap()
        for x_ in names
    }
    outT = nc.dram_tensor("outT", [b, e, s], mybir.dt.bfloat16, kind="ExternalOutput").ap()

    add = mybir.AluOpType.add
    bypass = mybir.AluOpType.bypass
    mult = mybir.AluOpType.mult
    sigmoid = mybir.ActivationFunctionType.Sigmoid
    identity = mybir.ActivationFunctionType.Identity
    bf16 = mybir.dt.bfloat16

    with tile.TileContext(nc) as tc, ExitStack() as ctx:
        wpool = ctx.enter_context(tc.tile_pool(name="w", bufs=1))
        cpool = ctx.enter_context(tc.tile_pool(name="const", bufs=1))
        xpool = ctx.enter_context(tc.tile_pool(name="x", bufs=3))
        ppool = ctx.enter_context(tc.tile_pool(name="psum", bufs=8, space="PSUM"))
        spool = ctx.enter_context(tc.tile_pool(name="work", bufs=5))
        opool = ctx.enter_context(tc.tile_pool(name="out", bufs=3))
        cspool = ctx.enter_context(tc.tile_pool(name="cs", bufs=6))

        # Biases via the SWDGE queue (parallel with the big HWDGE stream):
        # [128, mh], col m = bias[m*128:(m+1)*128]
        b_sb = {}
        for x_ in names:
            t_ = cpool.tile([128, mh], f32, tag=f"b{x_}")
            nc.gpsimd.dma_start(out=t_, in_=bias[x_].rearrange("(m p) -> p m", p=128))
            b_sb[x_] = t_

        def load_x(bi, n, n_dmas=1):
            # one whole-unit tile; n_dmas>1 splits the transfer so early
            # chunks land (and unblock matmuls) sooner
            t_ = xpool.tile([128, kc, tt], mm_dt, tag="xt")
            xsrc = X5[bi][n].rearrange("p j c t -> p (j c) t")
            step = kc // n_dmas
            for c0 in range(0, kc, step):
                nc.sync.dma_start(
                    out=t_[:, c0:c0 + step, :], in_=xsrc[:, c0:c0 + step, :]
                )
            return t_

        # Consumption-ordered prologue: unit (0,0)'s x per-chunk interleaved
        # with Wq chunks (the first chain's operands), then Wk/Wv/Wg, then
        # units (0,1)/(0,2) x.
        w_sb = {}
        for x_ in names:
            t_ = wpool.tile([128, kc, e], mm_dt, tag=f"w{x_}")
            w_sb[x_] = t_

        # single sync-queue prologue in consumption order: x(0,0) quarters
        # interleaved with Wq quarters, then Wk / x(0,1) / Wv / x(0,2) / Wg —
        # so units 1-2's x transfers don't queue behind all twelve W quarters
        # x(0,0) in 2-chunk eighths so the first matmul waits on only 256KB;
        # one Wk quarter after every second x eighth keeps issue pressure low
        # (starting even earlier with 1-chunk slices just moves the wait into
        # PE gaps — the cold DMA stream can't keep pace with the chain).
        # Weight order k,v,q,g matches the matmul chain order.
        x_first = xpool.tile([128, kc, tt], mm_dt, tag="xt")
        x0src = X5[0][0].rearrange("p j c t -> p (j c) t")
        for h in range(kc // 2):
            nc.sync.dma_start(
                out=x_first[:, 2 * h:2 * h + 2, :],
                in_=x0src[:, 2 * h:2 * h + 2, :],
            )
            if h % 2 == 0:
                j = h // 2
                nc.sync.dma_start(
                    out=w_sb["k"][:, j * XJ:(j + 1) * XJ, :],
                    in_=W5["k"][:, j * XJ:(j + 1) * XJ, :],
                )
        for x_ in "vqg":
            for j in range(NXJ):
                nc.sync.dma_start(
                    out=w_sb[x_][:, j * XJ:(j + 1) * XJ, :],
                    in_=W5[x_][:, j * XJ:(j + 1) * XJ, :],
                )

        def emit_unit(bi, n, xt, cs_prev, first_of_seq, vec_sub):
            """Full-width matmul chains for unit (bi, n); the ACT/DVE chain
            runs on vec_sub-wide psum slices (narrow for the final unit so
            the post-matmul drain is short)."""
            # k,v chains first: kv + the serial scan need only those psums,
            # so they hide under the q/g matmul chains
            ps = {}
            for m in range(mh):
                for x_ in "kvqg":
                    p_ = ppool.tile([128, tt], f32, tag="ps")
                    for c in range(kc):
                        nc.tensor.matmul(
                            p_[:],
                            lhsT=w_sb[x_][:, c, m * 128:(m + 1) * 128],
                            rhs=xt[:, c, :],
                            start=(c == 0),
                            stop=(c == kc - 1),
                        )
                    ps[x_, m] = p_

            for m in range(mh):
                # pass 1: k/v bias-adds, kv, scan per sub-slice (two-pass so
                # q/g ACT ops never block later slices' k/v in the queue)
                cs_list = []
                for t0 in range(0, tt, vec_sub):
                    tw = vec_sub
                    sl = slice(t0, t0 + tw)
                    k_sb = spool.tile([128, tw], bf16, tag="k")
                    nc.scalar.activation(
                        k_sb[:], ps["k", m][:, sl], identity,
                        bias=b_sb["k"][:, m:m + 1], scale=1.0,
                    )
                    v_sb = spool.tile([128, tw], bf16, tag="v")
                    nc.scalar.activation(
                        v_sb[:], ps["v", m][:, sl], identity,
                        bias=b_sb["v"][:, m:m + 1], scale=1.0,
                    )
                    kv = spool.tile([128, tw], bf16, tag="kv")
                    nc.vector.tensor_tensor(kv[:], k_sb[:], v_sb[:], mult)
                    cs = cspool.tile([128, tw], f32, tag="cs")
                    init = (0.0 if first_of_seq and t0 == 0
                            else cs_prev[m][:, -1:])
                    nc.vector.tensor_tensor_scan(
                        cs[:], kv[:], kv[:], init, op0=add, op1=bypass
                    )
                    cs_prev[m] = cs
                    cs_list.append(cs)
                # pass 2: q/g bias-adds, qg, output — sub-slices write into
                # one full-width tile so each m issues a single out-DMA
                # (per-slice DMAs cost ~600ns of queue issue time apiece,
                # which is exposed at the kernel tail)
                o_sb = opool.tile([128, tt], bf16, tag="o")
                q_list = []
                for t0 in range(0, tt, vec_sub):
                    sl = slice(t0, t0 + vec_sub)
                    q_sb = spool.tile([128, vec_sub], bf16, tag="q")
                    nc.scalar.activation(
                        q_sb[:], ps["q", m][:, sl], identity,
                        bias=b_sb["q"][:, m:m + 1], scale=1.0,
                    )
                    q_list.append(q_sb)
                for i, t0 in enumerate(range(0, tt, vec_sub)):
                    sl = slice(t0, t0 + vec_sub)
                    g_sb = spool.tile([128, vec_sub], bf16, tag="g")
                    nc.scalar.activation(
                        g_sb[:], ps["g", m][:, sl], sigmoid,
                        bias=b_sb["g"][:, m:m + 1], scale=1.0,
                    )
                    qg = spool.tile([128, vec_sub], bf16, tag="qg")
                    nc.gpsimd.tensor_tensor(qg[:], q_list[i][:], g_sb[:], mult)
                    nc.vector.tensor_tensor(o_sb[:, sl], qg[:], cs_list[i][:], mult)
                nc.sync.dma_start(
                    out=outT[bi][m * 128:(m + 1) * 128, n * tt:(n + 1) * tt],
                    in_=o_sb[:],
                )

        for bi in range(b):
            cs_prev = [None] * mh
            for n in range(nu):
                if bi == 0 and n == 0:
                    xt = x_first
                else:
                    # split loads keep chunks landing ahead of the matmul
                    # stream (a monolithic 2MB DMA starves the PE early on);
                    # steady-state units use halves (fewer, larger packets)
                    xt = load_x(bi, n, n_dmas=4 if bi == 0 else 2)
                last_unit = (bi == b - 1) and (n == nu - 1)
                emit_unit(bi, n, xt, cs_prev, first_of_seq=(n == 0),
                          vec_sub=128 if last_unit else tt)

    nc.compile()
    return nc


_NC_CACHE = {}


def _get_nc():
    if "nc" not in _NC_CACHE:
        _NC_CACHE["nc"] = build_nc()
    return _NC_CACHE["nc"]


# Zeroing low bf16 mantissa bits (RNE) was tested as a PE-power lever; it
# made no measurable difference to the DVFS util limit, so it stays off to
# preserve error margin.
TRUNC_K = 0


def _trunc_bf16_rne(a32, k=TRUNC_K):
    ab = a32.astype(MM_NP)
    if not k:
        return ab
    u = ab.view(np.uint16).astype(np.uint32)
    lsb = (u >> k) & 1
    u = (u + ((1 << (k - 1)) - 1 + lsb)) & (0xFFFF ^ ((1 << k) - 1))
    return u.astype(np.uint16).view(MM_NP)


def make_in_maps(x, Wq, bq, Wk, bk, Wv, bv, Wg, bg, e=E, n_cores=N_CORES):
    # X5[b, n, p, j, c, t] = x[b, n*TT+t, (j*XJ+c)*128+p]
    X5 = _trunc_bf16_rne(np.ascontiguousarray(
        np.asarray(x, dtype=np.float32)
        .reshape(B, NU, TT, NXJ, XJ, 128)
        .transpose(0, 1, 5, 3, 4, 2)
    ))
    Ws = {"q": Wq, "k": Wk, "v": Wv, "g": Wg}
    bs = {"q": bq, "k": bk, "v": bv, "g": bg}
    in_maps = []
    for core in range(n_cores):
        sl = slice(core * e, (core + 1) * e)
        m = {"X5": X5}
        for x_ in "qkvg":
            # W5[p, c, e] = W[sl][e, c*128+p]
            m[f"W{x_}5"] = _trunc_bf16_rne(np.ascontiguousarray(
                np.asarray(Ws[x_][sl, :], dtype=np.float32)
                .T.reshape(KC, 128, e)
                .transpose(1, 0, 2)
            ))
            m[f"b{x_}"] = np.ascontiguousarray(np.asarray(bs[x_][sl], dtype=np.float32))
        in_maps.append(m)
    return in_maps


def gather_out(results, n_cores=N_CORES):
    # each core returns outT [B, E, S] bf16; full out = [B, S, D] fp32
    outs = [r["outT"].astype(np.float32) for r in results]
    full = np.concatenate(outs, axis=1)  # [B, D, S]
    return np.ascontiguousarray(full.transpose(0, 2, 1))


def kernel(x, Wq, bq, Wk, bk, Wv, bv, Wg, bg, **run_kwargs):
    nc = _get_nc()
    in_maps = make_in_maps(x, Wq, bq, Wk, bk, Wv, bv, Wg, bg)
    res = run_bass_kernel_spmd(
        nc, in_maps, core_ids=list(range(N_CORES)), **run_kwargs
    )
    out = gather_out(res.results)
    if run_kwargs:
        _NC_CACHE["last_result"] = res
    return out



# revision 4
# speedup vs baseline: 1.0970x; 1.0970x over previous
# Trainium2 Bass kernel for:
#   q = x @ Wq.T + bq ; k = x @ Wk.T + bk ; v = x @ Wv.T + bv
#   g = sigmoid(x @ Wg.T + bg)
#   out = q * cumsum(k*v, axis=seq) * g
#
# Sharding: tensor-parallel split of the 2048 output features across the 8
# cores (256 features each). All ops are per-feature except the d-contraction
# (each core uses the full x) and the cumsum along seq (handled fully on-core
# per (batch, feature)) -> zero cross-core communication.
#
# v2 over the bf16 baseline (912.5us):
#   - g-chain runs NF8=12 of its 16 contraction chunks as fp8(e4m3)
#     DoubleRow matmuls (6 DR matmuls cover 256 contraction rows each at
#     ~2x rate) + 4 bf16 chunks. Numerics sim (exact datapath emulation vs
#     the jax reference) puts max-err/scale at 1.47e-2 vs the 2e-2 gate;
#     all-fp8 (4.7e-2) and fp8 on any other chain fail the gate, so only
#     g gets the treatment. The fp8 and bf16 partial sums share one PSUM
#     accumulator by pre-scaling x by 32 and Wg by 4096 (both exact
#     powers of 2) so every term carries the same 2^17 factor; the ACT
#     descales (1/32 for k,v,q via their existing bias-add, 2^-17 inside
#     g's sigmoid).
#   - fp32 downstream (k,v,q,g,kv,qg,out tiles + out DMA) halves the
#     rounding floor (6.7e-3 -> 3.1e-3), buying the fp8 error budget.
#   - ~44 dummy warmup matmuls on a zeroed tile keep the PE busy during
#     the DMA-bound prologue so the HAM clock gate opens (~1.2->2.4 GHz)
#     before the real matmul stream starts (was warming only at t=24us).
#   - W packed m-major ([128, MH, KC, 128]) so the first chain only
#     gates on x(0,0) + Wk's m=0 half; halves stream in consumption
#     order.
#
# On-core layout is [e, t] (features on partitions, tokens on the free dim):
#   - linears:  psum[e,t] += W_chunk.T @ x_chunk   (fp32 accum)
#   - bias:     ACT activation Identity with per-partition bias + descale
#   - sigmoid:  ACT activation with per-partition bias + descale
#   - cumsum:   DVE tensor_tensor_scan along the free dim (fp32),
#               chained across token (sub)tiles via initial=prev[:, -1:]
#   - qg mul on the Pool engine, kv/out muls on DVE.
# The final unit is processed in 128-token sub-tiles to shorten the
# post-matmul drain chain.

from contextlib import ExitStack

import numpy as np
import ml_dtypes

import concourse.bass as bass  # noqa: F401  (bass types referenced via tile/bacc)
import concourse.tile as tile
from concourse import bacc, mybir
from concourse.bass_utils import run_bass_kernel_spmd

N_CORES = 8
B, S, D = 4, 4096, 2048
E = D // N_CORES  # 256 output features per core
TT = 512          # token tile (free dim of psum)
KC = D // 128     # contraction chunks
NU = S // TT      # token tiles per batch
MH = E // 128     # feature halves (psum groups per linear)
MM_DT = mybir.dt.bfloat16
MM_NP = ml_dtypes.bfloat16
F8_DT = mybir.dt.float8e4
F8_NP = ml_dtypes.float8_e4m3  # TRN fp8e4: max normal 240, matches after clip

NF8 = 12                 # g-chain fp8 contraction chunks (must be even)
NBF = KC - NF8           # g-chain bf16 chunks
SX = 32.0                # x pre-scale (exact in bf16; uses e4m3 range)
SW = 4096.0              # Wg pre-scale (exact in bf16)
N_WARM = 44              # dummy warmup matmuls before the real stream


def build_nc(b=B, s=S, d=D, e=E, tt=TT, n_cores=N_CORES):
    kc = KC
    nu = NU
    mh = MH
    f32 = mybir.dt.float32
    names = "qkvg"

    nc = bacc.Bacc(
        "TRN2", target_bir_lowering=False, debug=False, num_devices=n_cores
    )
    # x packed on host (pre-scaled by SX): X5[b, n, p, c, t] = SX*x[b, n*tt+t, c*128+p]
    X5 = nc.dram_tensor(
        "X5", [b, nu, 128, kc, tt], MM_DT, kind="ExternalInput"
    ).ap()
    # fp8 copy of x's first NF8 chunks (same SX scale)
    X8 = nc.dram_tensor(
        "X8", [b, nu, 128, NF8, tt], F8_DT, kind="ExternalInput"
    ).ap()
    # W packed on host, m-major: W5[p, m, c, e'] = W[core_sl][m*128+e', c*128+p]
    W5 = {
        x_: nc.dram_tensor(f"W{x_}5", [128, mh, kc, 128], MM_DT, kind="ExternalInput").ap()
        for x_ in "qkv"
    }
    # g weights: fp8 chunks [0, NF8) scaled by SW, bf16 chunks [NF8, KC) scaled by SW
    Wg8 = nc.dram_tensor("Wg8", [128, mh, NF8, 128], F8_DT, kind="ExternalInput").ap()
    Wgb = nc.dram_tensor("Wgb", [128, mh, NBF, 128], MM_DT, kind="ExternalInput").ap()
    bias = {
        x_: nc.dram_tensor(f"b{x_}", [e], f32, kind="ExternalInput").ap()
        for x_ in names
    }
    outT = nc.dram_tensor("outT", [b, e, s], f32, kind="ExternalOutput").ap()

    add = mybir.AluOpType.add
    bypass = mybir.AluOpType.bypass
    mult = mybir.AluOpType.mult
    sigmoid = mybir.ActivationFunctionType.Sigmoid
    identity = mybir.ActivationFunctionType.Identity
    dr = mybir.MatmulPerfMode.DoubleRow

    with tile.TileContext(nc) as tc, ExitStack() as ctx:
        wpool = ctx.enter_context(tc.tile_pool(name="w", bufs=1))
        cpool = ctx.enter_context(tc.tile_pool(name="const", bufs=1))
        xpool = ctx.enter_context(tc.tile_pool(name="x", bufs=3))
        x8pool = ctx.enter_context(tc.tile_pool(name="x8", bufs=3))
        ppool = ctx.enter_context(tc.tile_pool(name="psum", bufs=8, space="PSUM"))
        spool = ctx.enter_context(tc.tile_pool(name="work", bufs=5))
        opool = ctx.enter_context(tc.tile_pool(name="out", bufs=3))
        cspool = ctx.enter_context(tc.tile_pool(name="cs", bufs=6))

        # Biases via the SWDGE queue (parallel with the big HWDGE stream):
        # [128, mh], col m = bias[m*128:(m+1)*128]
        b_sb = {}
        for x_ in names:
            t_ = cpool.tile([128, mh], f32, tag=f"b{x_}")
            nc.gpsimd.dma_start(out=t_, in_=bias[x_].rearrange("(m p) -> p m", p=128))
            b_sb[x_] = t_

        # PE warmup: dummy matmuls on a zeroed tile keep the PE's HAM
        # activity window busy while the prologue DMAs stream, so the
        # 1.2->2.4 GHz un-throttle fires before the real chains start.
        if N_WARM:
            dz = cpool.tile([128, 128], MM_DT, tag="warmz")
            nc.vector.memset(dz[:], 0.0)
            pd = ppool.tile([128, tt], f32, tag="ps")
            for _ in range(N_WARM):
                nc.tensor.matmul(
                    pd[:, :64], lhsT=dz[:], rhs=dz[:, :64], start=True, stop=True
                )

        w_sb = {}
        for x_ in "qkv":
            t_ = wpool.tile([128, mh, kc, 128], MM_DT, tag=f"w{x_}")
            w_sb[x_] = t_
        wg8_sb = wpool.tile([128, mh, NF8, 128], F8_DT, tag="wg8")
        wgb_sb = wpool.tile([128, mh, NBF, 128], MM_DT, tag="wgb")

        def load_x(bi, n, n_dmas=1):
            # bf16 unit tile + fp8 g-chunk tile; n_dmas>1 splits the bf16
            # transfer so early chunks land (and unblock matmuls) sooner
            t_ = xpool.tile([128, kc, tt], MM_DT, tag="xt")
            step = kc // n_dmas
            for c0 in range(0, kc, step):
                nc.sync.dma_start(
                    out=t_[:, c0:c0 + step, :], in_=X5[bi][n][:, c0:c0 + step, :]
                )
            t8 = x8pool.tile([128, NF8, tt], F8_DT, tag="x8t")
            nc.sync.dma_start(out=t8, in_=X8[bi][n])
            return t_, t8

        # Consumption-ordered prologue on the sync queue: x(0,0) eighths
        # interleaved with Wk-m0 quarters (the first chain's operands), then
        # the remaining m0 weight halves in chain order (v, q, g), x8(0,0),
        # then the m1 halves, then units (0,1)/(0,2).
        x_first = xpool.tile([128, kc, tt], MM_DT, tag="xt")
        for h in range(kc // 2):
            nc.sync.dma_start(
                out=x_first[:, 2 * h:2 * h + 2, :],
                in_=X5[0][0][:, 2 * h:2 * h + 2, :],
            )
            if h < 4:
                nc.sync.dma_start(
                    out=w_sb["k"][:, 0, 4 * h:4 * h + 4, :],
                    in_=W5["k"][:, 0, 4 * h:4 * h + 4, :],
                )
        nc.sync.dma_start(out=w_sb["v"][:, 0], in_=W5["v"][:, 0])
        nc.sync.dma_start(out=w_sb["q"][:, 0], in_=W5["q"][:, 0])
        nc.sync.dma_start(out=wg8_sb[:, 0], in_=Wg8[:, 0])
        nc.sync.dma_start(out=wgb_sb[:, 0], in_=Wgb[:, 0])
        x8_first = x8pool.tile([128, NF8, tt], F8_DT, tag="x8t")
        nc.sync.dma_start(out=x8_first, in_=X8[0][0])
        for x_ in "kvq":
            nc.sync.dma_start(out=w_sb[x_][:, 1], in_=W5[x_][:, 1])
        nc.sync.dma_start(out=wg8_sb[:, 1], in_=Wg8[:, 1])
        nc.sync.dma_start(out=wgb_sb[:, 1], in_=Wgb[:, 1])

        def emit_unit(bi, n, xt, x8t, cs_prev, first_of_seq, vec_sub):
            """Full-width matmul chains for unit (bi, n); the ACT/DVE chain
            runs on vec_sub-wide psum slices (narrow for the final unit so
            the post-matmul drain is short)."""
            # k,v chains first: kv + the serial scan need only those psums,
            # so they hide under the q/g matmul chains
            ps = {}
            for m in range(mh):
                for x_ in "kvq":
                    p_ = ppool.tile([128, tt], f32, tag="ps")
                    for c in range(kc):
                        nc.tensor.matmul(
                            p_[:],
                            lhsT=w_sb[x_][:, m, c, :],
                            rhs=xt[:, c, :],
                            start=(c == 0),
                            stop=(c == kc - 1),
                        )
                    ps[x_, m] = p_
                # g: fp8 DoubleRow pairs (256 contraction rows per MM), then
                # bf16 chunks, accumulating into one psum (common 2^17 scale)
                p_ = ppool.tile([128, tt], f32, tag="ps")
                for j in range(NF8 // 2):
                    nc.tensor.matmul(
                        p_[:],
                        lhsT=wg8_sb[:, m, 2 * j:2 * j + 2, :],
                        rhs=x8t[:, 2 * j:2 * j + 2, :],
                        start=(j == 0),
                        stop=False,
                        perf_mode=dr,
                    )
                for cb in range(NBF):
                    nc.tensor.matmul(
                        p_[:],
                        lhsT=wgb_sb[:, m, cb, :],
                        rhs=xt[:, NF8 + cb, :],
                        start=False,
                        stop=(cb == NBF - 1),
                    )
                ps["g", m] = p_

            for m in range(mh):
                # pass 1: k/v bias-adds (descale 1/SX), kv, scan per
                # sub-slice (two-pass so q/g ACT ops never block later
                # slices' k/v in the queue)
                cs_list = []
                for t0 in range(0, tt, vec_sub):
                    tw = vec_sub
                    sl = slice(t0, t0 + tw)
                    k_sb = spool.tile([128, tw], f32, tag="k")
                    nc.scalar.activation(
                        k_sb[:], ps["k", m][:, sl], identity,
                        bias=b_sb["k"][:, m:m + 1], scale=1.0 / SX,
                    )
                    v_sb = spool.tile([128, tw], f32, tag="v")
                    nc.scalar.activation(
                        v_sb[:], ps["v", m][:, sl], identity,
                        bias=b_sb["v"][:, m:m + 1], scale=1.0 / SX,
                    )
                    kv = spool.tile([128, tw], f32, tag="kv")
                    nc.vector.tensor_tensor(kv[:], k_sb[:], v_sb[:], mult)
                    cs = cspool.tile([128, tw], f32, tag="cs")
                    init = (0.0 if first_of_seq and t0 == 0
                            else cs_prev[m][:, -1:])
                    nc.vector.tensor_tensor_scan(
                        cs[:], kv[:], kv[:], init, op0=add, op1=bypass
                    )
                    cs_prev[m] = cs
                    cs_list.append(cs)
                # pass 2: q/g bias-adds, qg, output — sub-slices write into
                # one full-width tile so each m issues a single out-DMA
                o_sb = opool.tile([128, tt], f32, tag="o")
                q_list = []
                for t0 in range(0, tt, vec_sub):
                    sl = slice(t0, t0 + vec_sub)
                    q_sb = spool.tile([128, vec_sub], f32, tag="q")
                    nc.scalar.activation(
                        q_sb[:], ps["q", m][:, sl], identity,
                        bias=b_sb["q"][:, m:m + 1], scale=1.0 / SX,
                    )
                    q_list.append(q_sb)
                for i, t0 in enumerate(range(0, tt, vec_sub)):
                    sl = slice(t0, t0 + vec_sub)
                    g_sb = spool.tile([128, vec_sub], f32, tag="g")
                    nc.scalar.activation(
                        g_sb[:], ps["g", m][:, sl], sigmoid,
                        bias=b_sb["g"][:, m:m + 1], scale=1.0 / (SX * SW),
                    )
                    qg = spool.tile([128, vec_sub], f32, tag="qg")
                    nc.gpsimd.tensor_tensor(qg[:], q_list[i][:], g_sb[:], mult)
                    nc.vector.tensor_tensor(o_sb[:, sl], qg[:], cs_list[i][:], mult)
                nc.sync.dma_start(
                    out=outT[bi][m * 128:(m + 1) * 128, n * tt:(n + 1) * tt],
                    in_=o_sb[:],
                )

        for bi in range(b):
            cs_prev = [None] * mh
            for n in range(nu):
                if bi == 0 and n == 0:
                    xt, x8t = x_first, x8_first
                else:
                    # split loads keep chunks landing ahead of the matmul
                    # stream (a monolithic 2MB DMA starves the PE early on)
                    xt, x8t = load_x(bi, n, n_dmas=4 if bi == 0 else 2)
                last_unit = (bi == b - 1) and (n == nu - 1)
                emit_unit(bi, n, xt, x8t, cs_prev, first_of_seq=(n == 0),
                          vec_sub=128 if last_unit else tt)

    nc.compile()
    return nc


_NC_CACHE = {}


def _get_nc():
    if "nc" not in _NC_CACHE:
        _NC_CACHE["nc"] = build_nc()
    return _NC_CACHE["nc"]


def make_in_maps(x, Wq, bq, Wk, bk, Wv, bv, Wg, bg, e=E, n_cores=N_CORES):
    xs = np.asarray(x, dtype=np.float32) * SX
    # X5[b, n, p, c, t] = SX*x[b, n*TT+t, c*128+p]
    xt = xs.reshape(B, NU, TT, KC, 128).transpose(0, 1, 4, 3, 2)
    X5 = np.ascontiguousarray(xt).astype(MM_NP)
    X8 = np.clip(np.ascontiguousarray(xt[:, :, :, :NF8, :]), -240, 240).astype(F8_NP)
    Ws = {"q": Wq, "k": Wk, "v": Wv, "g": Wg}
    bs = {"q": bq, "k": bk, "v": bv, "g": bg}
    in_maps = []
    for core in range(n_cores):
        sl = slice(core * e, (core + 1) * e)
        m = {"X5": X5, "X8": X8}
        for x_ in "qkv":
            # W5[p, m, c, e'] = W[sl][m*128+e', c*128+p]
            m[f"W{x_}5"] = np.ascontiguousarray(
                np.asarray(Ws[x_][sl, :], dtype=np.float32)
                .T.reshape(KC, 128, MH, 128)
                .transpose(1, 2, 0, 3)
            ).astype(MM_NP)
        wg = np.asarray(Wg[sl, :], dtype=np.float32).T * SW
        wg = wg.reshape(KC, 128, MH, 128).transpose(1, 2, 0, 3)
        m["Wg8"] = np.clip(
            np.ascontiguousarray(wg[:, :, :NF8, :]), -240, 240
        ).astype(F8_NP)
        m["Wgb"] = np.ascontiguousarray(wg[:, :, NF8:, :]).astype(MM_NP)
        for x_ in "qkvg":
            m[f"b{x_}"] = np.ascontiguousarray(np.asarray(bs[x_][sl], dtype=np.float32))
        in_maps.append(m)
    return in_maps


def gather_out(results, n_cores=N_CORES):
    # each core returns outT [B, E, S] f32; full out = [B, S, D] f32
    outs = [r["outT"] for r in results]
    full = np.concatenate(outs, axis=1)  # [B, D, S]
    return np.ascontiguousarray(full.transpose(0, 2, 1).astype(np.float32))


def kernel(x, Wq, bq, Wk, bk, Wv, bv, Wg, bg, **run_kwargs):
    nc = _get_nc()
    in_maps = make_in_maps(x, Wq, bq, Wk, bk, Wv, bv, Wg, bg)
    res = run_bass_kernel_spmd(
        nc, in_maps, core_ids=list(range(N_CORES)), **run_kwargs
    )
    out = gather_out(res.results)
    if run_kwargs:
        _NC_CACHE["last_result"] = res
    return out


# revision 5
# speedup vs baseline: 1.1230x; 1.0237x over previous
# Trainium2 Bass kernel for:
#   q = x @ Wq.T + bq ; k = x @ Wk.T + bk ; v = x @ Wv.T + bv
#   g = sigmoid(x @ Wg.T + bg)
#   out = q * cumsum(k*v, axis=seq) * g
#
# Sharding: tensor-parallel split of the 2048 output features across the 8
# cores (256 features each). All ops are per-feature except the d-contraction
# (each core uses the full x) and the cumsum along seq (handled fully on-core
# per (batch, feature)) -> zero cross-core communication.
#
# v3 over the bf16 baseline (912.5us):
#   - Part of the contraction runs as fp8(e4m3) DoubleRow matmuls: 14 of
#     g's 16 chunks and 2 of v's (numerics sim vs the jax reference —
#     which matched HW to 4 digits on two configs — puts max-err/scale
#     at 1.57e-2 vs the 2e-2 gate; more fp8 anywhere crosses 1.9e-2).
#     Warm DR matmuls stream 256 contraction rows in the same 216ns a
#     bf16 matmul needs for 128 rows (full 2x; LDW hidden).
#   - The fp8 and bf16 partial sums share one PSUM accumulator by
#     pre-scaling x by 32 and Wv/Wg by 4096 (exact powers of 2); the ACT
#     descales (1/32 for k,q; 2^-17 for v,g) in the existing bias-add.
#   - DR->bf16 mode switches cost ~220ns, so the v-DR and g-DR sections
#     are emitted back-to-back (chain order k,q,v,g with v's DR last and
#     g's DR first) -> one switch per m-group instead of two.
#   - fp32 downstream (k,v,q,g,kv,qg,out + out DMA) halves the rounding
#     floor (6.7e-3 -> 3.1e-3), buying the fp8 error budget.
#   - 110 dummy warmup matmuls on a zeroed tile keep the PE busy from the
#     end of the ~7.4us framework init so the HAM clock gate opens
#     (1.2 -> 2.4 GHz) during the DMA-bound prologue, not at t=26us.
#   - W packed m-major ([128, MH, KC, 128]) so the first chain only
#     gates on x(0,0) + Wk's m=0 half; halves stream in consumption
#     order; x8 rides the scalar SWDGE queue in parallel with the big
#     sync-queue stream.
#
# On-core layout is [e, t] (features on partitions, tokens on the free dim):
#   - linears:  psum[e,t] += W_chunk.T @ x_chunk   (fp32 accum)
#   - bias:     ACT activation Identity with per-partition bias + descale
#   - sigmoid:  ACT activation with per-partition bias + descale
#   - cumsum:   DVE tensor_tensor_scan along the free dim (fp32),
#               chained across token (sub)tiles via initial=prev[:, -1:]
#   - qg mul on the Pool engine, kv/out muls on DVE.
# The final unit is processed in 128-token sub-tiles to shorten the
# post-matmul drain chain.

from contextlib import ExitStack

import numpy as np
import ml_dtypes

import concourse.bass as bass  # noqa: F401  (bass types referenced via tile/bacc)
import concourse.tile as tile
from concourse import bacc, mybir
from concourse.bass_utils import run_bass_kernel_spmd

N_CORES = 8
B, S, D = 4, 4096, 2048
E = D // N_CORES  # 256 output features per core
TT = 512          # token tile (free dim of psum)
KC = D // 128     # contraction chunks
NU = S // TT      # token tiles per batch
MH = E // 128     # feature halves (psum groups per linear)
MM_DT = mybir.dt.bfloat16
MM_NP = ml_dtypes.bfloat16
F8_DT = mybir.dt.float8e4
F8_NP = ml_dtypes.float8_e4m3  # TRN fp8e4: max normal 240, matches after clip

# per-chain fp8 contraction chunks (each must be even; fp8 covers the FIRST
# nf8 chunks of that chain's contraction)
NF8 = {"q": 0, "k": 0, "v": 2, "g": 14}
NX8 = max(NF8.values())  # chunks of x kept in fp8
SX = 32.0                # x pre-scale (exact in bf16; uses e4m3 range)
SW = 4096.0              # W pre-scale for chains with fp8 chunks
N_WARM = 110             # dummy warmup matmuls before the real stream


def build_nc(b=B, s=S, d=D, e=E, tt=TT, n_cores=N_CORES):
    kc = KC
    nu = NU
    mh = MH
    f32 = mybir.dt.float32
    names = "qkvg"

    nc = bacc.Bacc(
        "TRN2", target_bir_lowering=False, debug=False, num_devices=n_cores
    )
    # x packed on host (pre-scaled by SX): X5[b, n, p, c, t] = SX*x[b, n*tt+t, c*128+p]
    X5 = nc.dram_tensor(
        "X5", [b, nu, 128, kc, tt], MM_DT, kind="ExternalInput"
    ).ap()
    # fp8 copy of x's first NX8 chunks (same SX scale)
    X8 = nc.dram_tensor(
        "X8", [b, nu, 128, NX8, tt], F8_DT, kind="ExternalInput"
    ).ap()
    # W packed on host, m-major: [p, m, c, e'] = W[core_sl][m*128+e', c*128+p]
    # chains with fp8 chunks ship W (x SW) as a fp8 part + a bf16 part
    Wb = {}
    W8 = {}
    for x_ in names:
        n8 = NF8[x_]
        Wb[x_] = nc.dram_tensor(
            f"W{x_}b", [128, mh, kc - n8, 128], MM_DT, kind="ExternalInput"
        ).ap()
        if n8:
            W8[x_] = nc.dram_tensor(
                f"W{x_}8", [128, mh, n8, 128], F8_DT, kind="ExternalInput"
            ).ap()
    bias = {
        x_: nc.dram_tensor(f"b{x_}", [e], f32, kind="ExternalInput").ap()
        for x_ in names
    }
    outT = nc.dram_tensor("outT", [b, e, s], f32, kind="ExternalOutput").ap()

    add = mybir.AluOpType.add
    bypass = mybir.AluOpType.bypass
    mult = mybir.AluOpType.mult
    sigmoid = mybir.ActivationFunctionType.Sigmoid
    identity = mybir.ActivationFunctionType.Identity
    dr = mybir.MatmulPerfMode.DoubleRow
    descale = {x_: 1.0 / (SX * (SW if NF8[x_] else 1.0)) for x_ in names}

    with tile.TileContext(nc) as tc, ExitStack() as ctx:
        wpool = ctx.enter_context(tc.tile_pool(name="w", bufs=1))
        cpool = ctx.enter_context(tc.tile_pool(name="const", bufs=1))
        xpool = ctx.enter_context(tc.tile_pool(name="x", bufs=3))
        x8pool = ctx.enter_context(tc.tile_pool(name="x8", bufs=3))
        ppool = ctx.enter_context(tc.tile_pool(name="psum", bufs=8, space="PSUM"))
        spool = ctx.enter_context(tc.tile_pool(name="work", bufs=5))
        opool = ctx.enter_context(tc.tile_pool(name="out", bufs=3))
        cspool = ctx.enter_context(tc.tile_pool(name="cs", bufs=6))

        # Biases via the gpsimd SWDGE queue (parallel with the sync stream):
        # [128, mh], col m = bias[m*128:(m+1)*128]
        b_sb = {}
        for x_ in names:
            t_ = cpool.tile([128, mh], f32, tag=f"b{x_}")
            nc.gpsimd.dma_start(out=t_, in_=bias[x_].rearrange("(m p) -> p m", p=128))
            b_sb[x_] = t_

        # PE warmup: dummy matmuls on a zeroed tile keep the PE's HAM
        # activity window busy while the prologue DMAs stream, so the
        # 1.2->2.4 GHz un-throttle fires before the real chains start.
        if N_WARM:
            dz = cpool.tile([128, 128], MM_DT, tag="warmz")
            nc.vector.memset(dz[:], 0.0)
            pd = ppool.tile([128, tt], f32, tag="ps")
            for _ in range(N_WARM):
                nc.tensor.matmul(
                    pd[:, :64], lhsT=dz[:], rhs=dz[:, :64], start=True, stop=True
                )

        wb_sb = {}
        w8_sb = {}
        for x_ in names:
            n8 = NF8[x_]
            t_ = wpool.tile([128, mh, kc - n8, 128], MM_DT, tag=f"w{x_}b")
            wb_sb[x_] = t_
            if n8:
                t8_ = wpool.tile([128, mh, n8, 128], F8_DT, tag=f"w{x_}8")
                w8_sb[x_] = t8_

        def load_x(bi, n, n_dmas=1):
            # bf16 unit tile (sync queue) + fp8 chunk tile (scalar queue);
            # n_dmas>1 splits the bf16 transfer so early chunks land (and
            # unblock matmuls) sooner
            t_ = xpool.tile([128, kc, tt], MM_DT, tag="xt")
            step = kc // n_dmas
            for c0 in range(0, kc, step):
                nc.sync.dma_start(
                    out=t_[:, c0:c0 + step, :], in_=X5[bi][n][:, c0:c0 + step, :]
                )
            t8 = x8pool.tile([128, NX8, tt], F8_DT, tag="x8t")
            nc.scalar.dma_start(out=t8, in_=X8[bi][n])
            return t_, t8

        def load_w_half(x_, m):
            # one m-half of a chain's weights, in chain consumption order
            nc.sync.dma_start(out=wb_sb[x_][:, m], in_=Wb[x_][:, m])
            if NF8[x_]:
                nc.sync.dma_start(out=w8_sb[x_][:, m], in_=W8[x_][:, m])

        # Consumption-ordered prologue on the sync queue: x(0,0) eighths
        # interleaved with Wk-m0 quarters (the first chain's operands), then
        # the remaining m0 weight halves in chain order (q, v, g), then the
        # m1 halves, then units (0,1)/(0,2). x8(0,0) rides the scalar queue
        # in parallel.
        x_first = xpool.tile([128, kc, tt], MM_DT, tag="xt")
        x8_first = x8pool.tile([128, NX8, tt], F8_DT, tag="x8t")
        nc.scalar.dma_start(out=x8_first, in_=X8[0][0])
        for h in range(kc // 2):
            nc.sync.dma_start(
                out=x_first[:, 2 * h:2 * h + 2, :],
                in_=X5[0][0][:, 2 * h:2 * h + 2, :],
            )
            if h < 4:
                nc.sync.dma_start(
                    out=wb_sb["k"][:, 0, 4 * h:4 * h + 4, :],
                    in_=Wb["k"][:, 0, 4 * h:4 * h + 4, :],
                )
        for x_ in "qvg":
            load_w_half(x_, 0)
        for x_ in "kqvg":
            load_w_half(x_, 1)

        def emit_unit(bi, n, xt, x8t, cs_prev, first_of_seq, vec_sub):
            """Full-width matmul chains for unit (bi, n); the ACT/DVE chain
            runs on vec_sub-wide psum slices (narrow for the final unit so
            the post-matmul drain is short)."""
            # chain order k,q,v,g: k early for the kv/scan chain, and v's
            # trailing DR section lands adjacent to g's leading one so the
            # DR->bf16 mode switch is paid once per m-group
            ps = {}
            for m in range(mh):
                for x_ in "kq":
                    p_ = ppool.tile([128, tt], f32, tag="ps")
                    for c in range(kc):
                        nc.tensor.matmul(
                            p_[:],
                            lhsT=wb_sb[x_][:, m, c, :],
                            rhs=xt[:, c, :],
                            start=(c == 0),
                            stop=(c == kc - 1),
                        )
                    ps[x_, m] = p_
                # v: bf16 chunks [nv8, kc) first, then DR pairs [0, nv8)
                nv8 = NF8["v"]
                p_ = ppool.tile([128, tt], f32, tag="ps")
                for i in range(kc - nv8):
                    nc.tensor.matmul(
                        p_[:],
                        lhsT=wb_sb["v"][:, m, i, :],
                        rhs=xt[:, nv8 + i, :],
                        start=(i == 0),
                        stop=False,
                    )
                for j in range(nv8 // 2):
                    nc.tensor.matmul(
                        p_[:],
                        lhsT=w8_sb["v"][:, m, 2 * j:2 * j + 2, :],
                        rhs=x8t[:, 2 * j:2 * j + 2, :],
                        start=False,
                        stop=(j == nv8 // 2 - 1),
                        perf_mode=dr,
                    )
                ps["v", m] = p_
                # g: DR pairs [0, ng8) first, then bf16 chunks [ng8, kc)
                ng8 = NF8["g"]
                p_ = ppool.tile([128, tt], f32, tag="ps")
                for j in range(ng8 // 2):
                    nc.tensor.matmul(
                        p_[:],
                        lhsT=w8_sb["g"][:, m, 2 * j:2 * j + 2, :],
                        rhs=x8t[:, 2 * j:2 * j + 2, :],
                        start=(j == 0),
                        stop=False,
                        perf_mode=dr,
                    )
                for i in range(kc - ng8):
                    nc.tensor.matmul(
                        p_[:],
                        lhsT=wb_sb["g"][:, m, i, :],
                        rhs=xt[:, ng8 + i, :],
                        start=False,
                        stop=(i == kc - ng8 - 1),
                    )
                ps["g", m] = p_

            for m in range(mh):
                # pass 1: k/v bias-adds (+descale), kv, scan per sub-slice
                # (two-pass so q/g ACT ops never block later slices' k/v in
                # the queue)
                cs_list = []
                for t0 in range(0, tt, vec_sub):
                    tw = vec_sub
                    sl = slice(t0, t0 + tw)
                    k_sb = spool.tile([128, tw], f32, tag="k")
                    nc.scalar.activation(
                        k_sb[:], ps["k", m][:, sl], identity,
                        bias=b_sb["k"][:, m:m + 1], scale=descale["k"],
                    )
                    v_sb = spool.tile([128, tw], f32, tag="v")
                    nc.scalar.activation(
                        v_sb[:], ps["v", m][:, sl], identity,
                        bias=b_sb["v"][:, m:m + 1], scale=descale["v"],
                    )
                    kv = spool.tile([128, tw], f32, tag="kv")
                    nc.vector.tensor_tensor(kv[:], k_sb[:], v_sb[:], mult)
                    cs = cspool.tile([128, tw], f32, tag="cs")
                    init = (0.0 if first_of_seq and t0 == 0
                            else cs_prev[m][:, -1:])
                    nc.vector.tensor_tensor_scan(
                        cs[:], kv[:], kv[:], init, op0=add, op1=bypass
                    )
                    cs_prev[m] = cs
                    cs_list.append(cs)
                # pass 2: q/g bias-adds, qg, output — sub-slices write into
                # one full-width tile so each m issues a single out-DMA
                o_sb = opool.tile([128, tt], f32, tag="o")
                q_list = []
                for t0 in range(0, tt, vec_sub):
                    sl = slice(t0, t0 + vec_sub)
                    q_sb = spool.tile([128, vec_sub], f32, tag="q")
                    nc.scalar.activation(
                        q_sb[:], ps["q", m][:, sl], identity,
                        bias=b_sb["q"][:, m:m + 1], scale=descale["q"],
                    )
                    q_list.append(q_sb)
                for i, t0 in enumerate(range(0, tt, vec_sub)):
                    sl = slice(t0, t0 + vec_sub)
                    g_sb = spool.tile([128, vec_sub], f32, tag="g")
                    nc.scalar.activation(
                        g_sb[:], ps["g", m][:, sl], sigmoid,
                        bias=b_sb["g"][:, m:m + 1], scale=descale["g"],
                    )
                    qg = spool.tile([128, vec_sub], f32, tag="qg")
                    nc.gpsimd.tensor_tensor(qg[:], q_list[i][:], g_sb[:], mult)
                    nc.vector.tensor_tensor(o_sb[:, sl], qg[:], cs_list[i][:], mult)
                nc.sync.dma_start(
                    out=outT[bi][m * 128:(m + 1) * 128, n * tt:(n + 1) * tt],
                    in_=o_sb[:],
                )

        for bi in range(b):
            cs_prev = [None] * mh
            for n in range(nu):
                if bi == 0 and n == 0:
                    xt, x8t = x_first, x8_first
                else:
                    # split loads keep chunks landing ahead of the matmul
                    # stream (a monolithic 2MB DMA starves the PE early on)
                    xt, x8t = load_x(bi, n, n_dmas=4 if bi == 0 else 2)
                last_unit = (bi == b - 1) and (n == nu - 1)
                emit_unit(bi, n, xt, x8t, cs_prev, first_of_seq=(n == 0),
                          vec_sub=128 if last_unit else tt)

    nc.compile()
    return nc


_NC_CACHE = {}


def _get_nc():
    if "nc" not in _NC_CACHE:
        _NC_CACHE["nc"] = build_nc()
    return _NC_CACHE["nc"]


def make_in_maps(x, Wq, bq, Wk, bk, Wv, bv, Wg, bg, e=E, n_cores=N_CORES):
    xs = np.asarray(x, dtype=np.float32) * SX
    # X5[b, n, p, c, t] = SX*x[b, n*TT+t, c*128+p]
    xt = xs.reshape(B, NU, TT, KC, 128).transpose(0, 1, 4, 3, 2)
    X5 = np.ascontiguousarray(xt).astype(MM_NP)
    X8 = np.clip(np.ascontiguousarray(xt[:, :, :, :NX8, :]), -240, 240).astype(F8_NP)
    Ws = {"q": Wq, "k": Wk, "v": Wv, "g": Wg}
    bs = {"q": bq, "k": bk, "v": bv, "g": bg}
    in_maps = []
    for core in range(n_cores):
        sl = slice(core * e, (core + 1) * e)
        m = {"X5": X5, "X8": X8}
        for x_ in "qkvg":
            n8 = NF8[x_]
            # [p, m, c, e'] = W[sl][m*128+e', c*128+p] (x SW for fp8 chains)
            w = np.asarray(Ws[x_][sl, :], dtype=np.float32).T
            if n8:
                w = w * SW
            w = w.reshape(KC, 128, MH, 128).transpose(1, 2, 0, 3)
            m[f"W{x_}b"] = np.ascontiguousarray(w[:, :, n8:, :]).astype(MM_NP)
            if n8:
                m[f"W{x_}8"] = np.clip(
                    np.ascontiguousarray(w[:, :, :n8, :]), -240, 240
                ).astype(F8_NP)
            m[f"b{x_}"] = np.ascontiguousarray(np.asarray(bs[x_][sl], dtype=np.float32))
        in_maps.append(m)
    return in_maps


def gather_out(results, n_cores=N_CORES):
    # each core returns outT [B, E, S] f32; full out = [B, S, D] f32
    outs = [r["outT"] for r in results]
    full = np.concatenate(outs, axis=1)  # [B, D, S]
    return np.ascontiguousarray(full.transpose(0, 2, 1).astype(np.float32))


def kernel(x, Wq, bq, Wk, bk, Wv, bv, Wg, bg, **run_kwargs):
    nc = _get_nc()
    in_maps = make_in_maps(x, Wq, bq, Wk, bk, Wv, bv, Wg, bg)
    res = run_bass_kernel_spmd(
        nc, in_maps, core_ids=list(range(N_CORES)), **run_kwargs
    )
    out = gather_out(res.results)
    if run_kwargs:
        _NC_CACHE["last_result"] = res
    return out
